# revision 1
# baseline (speedup 1.0000x reference)
"""Trainium2 Bass kernel for nn_CA_Module (channel-attention + SE gating).

Reference computation per sample (C=512, N=H*W=4096):
    q = x.reshape(C, N)
    energy = q @ q.T                     # [C, C]
    att = softmax(max_row - energy)      # == softmax(-energy)  (row shift cancels)
        -> G = exp(min_row - energy); att = G / rowsum(G)
    out = att @ q                        # [C, N]
    pooled = concat([mean_n(x), mean_n(out)])        # [2C]
    h  = relu(w1 @ pooled + b1)                      # [64]
    se = sigmoid(w2 @ h + b2)                        # [C]
    y  = se * x + (1 - se) * out

Key algebraic tricks used here:
  * softmax(max-e) == softmax(-e): compute G = exp(min_row - e) directly.
  * energy is symmetric, so G^T (needed as the stationary operand of the
    second matmul) is obtained by 16 cheap PE tile-transposes of G.
  * out = diag(1/S) (G @ q), so normalization folds into the final blend:
        y = se*x + beta*(G@q),  beta = (1-se)/S
  * mean_n(out) = G @ mean_n(x) / S  -- a tiny matvec, so the SE gate is
    known *before* the big second matmul and the blend fuses into PSUM
    evacuation.
  * matmuls run as float32r (full fp32 data, reduced-precision PE mode,
    1 cycle/row at free-dim >= 256 -- same speed as bf16).

Sharding: data-parallel over batch, 2 samples per core on 8 cores.
"""

import numpy as np

try:
    import concourse.bass as bass
except ImportError:
    import sys

    sys.path.insert(0, "/opt/trn_rl_repo")
    import concourse.bass as bass

import concourse.tile as tile
from concourse import bacc, mybir
from concourse import bass_utils as _bu
from concourse.bass_utils import run_bass_kernel_spmd
from concourse.masks import make_identity

# Enable walrus's weight-load optimization (background-buffer LDW overlap /
# dedup). The concourse default passes --enable-ldw-opt=false; measured on
# hardware this costs ~2x on 4-byte matmul streams, and enabling it is
# numerically verified on this kernel.
if not getattr(_bu, "_ldw_opt_patched", False):
    _orig_run_command = _bu.run_command

    def _run_command_ldw(cmd, *a, **k):
        if isinstance(cmd, list):
            cmd = [
                "--enable-ldw-opt=true" if c == "--enable-ldw-opt=false" else c
                for c in cmd
            ]
        return _orig_run_command(cmd, *a, **k)

    _bu.run_command = _run_command_ldw
    _bu._ldw_opt_patched = True

F32 = mybir.dt.float32
F32R = mybir.dt.float32r
AF = mybir.ActivationFunctionType
ALU = mybir.AluOpType
AX = mybir.AxisListType

B_TOTAL = 16
N_CORES = 8
B_PER_CORE = B_TOTAL // N_CORES  # 2
C = 512
N = 4096
CB = C // 128  # 4 c-blocks
KT = N // 128  # 32 n-slices for transpose/mm1
NCH = N // 512  # 8 n-chunks for mm2


def _build_program(reps: int = 1) -> bass.Bass:
    nc = bacc.Bacc(target_bir_lowering=False, debug=False)

    x_d = nc.dram_tensor("x", [B_PER_CORE, C, N], F32, kind="ExternalInput").ap()
    w1_d = nc.dram_tensor("w1", [64, 2 * C], F32, kind="ExternalInput").ap()
    b1_d = nc.dram_tensor("b1", [64, 1], F32, kind="ExternalInput").ap()
    w2_d = nc.dram_tensor("w2", [C, 64], F32, kind="ExternalInput").ap()
    b2_d = nc.dram_tensor("b2", [C, 1], F32, kind="ExternalInput").ap()
    y_d = nc.dram_tensor("y", [B_PER_CORE, C, N], F32, kind="ExternalOutput").ap()

    with tile.TileContext(nc) as tc:
        _emit(tc, x_d, w1_d, b1_d, w2_d, b2_d, y_d, reps)
    nc.compile()
    return nc


def _emit(tc, x_d, w1_d, b1_d, w2_d, b2_d, y_d, reps=1):
    nc = tc.nc
    from contextlib import ExitStack

    with ExitStack() as ctx:
        singles = ctx.enter_context(tc.tile_pool(name="singles", bufs=1))
        qpool = ctx.enter_context(tc.tile_pool(name="qpool", bufs=2))
        qtpool = ctx.enter_context(tc.tile_pool(name="qtpool", bufs=4))
        gpool = ctx.enter_context(tc.tile_pool(name="gpool", bufs=1))
        gtpool = ctx.enter_context(tc.tile_pool(name="gtpool", bufs=2))
        stats = ctx.enter_context(tc.tile_pool(name="stats", bufs=2))
        outp = ctx.enter_context(tc.tile_pool(name="outp", bufs=3))
        psum = ctx.enter_context(tc.tile_pool(name="psum", bufs=1, space="PSUM"))

        # ---- one-time setup -------------------------------------------------
        ident = singles.tile([128, 128], F32)
        make_identity(nc, ident)
        ident_r = singles.tile([128, 128], F32R)
        nc.vector.tensor_copy(ident_r, ident)
        # warm-up transposes: absorb the identity-producer waits into the PE
        # clock so later transposes carry at most one (DMA) wait
        warm = psum.tile([128, 128], F32, tag="tstage", bufs=3)
        nc.tensor.transpose(warm, ident, ident)
        warm2 = psum.tile([128, 128], F32, tag="tstage", bufs=3)
        nc.tensor.transpose(warm2.bitcast(F32R), ident_r, ident_r)

        # w1T: [k=2C partitions over 8 tiles, m=64] packed as [128, 8*64]
        w1_nat = singles.tile([64, 2 * C], F32)
        nc.sync.dma_start(out=w1_nat, in_=w1_d)
        w1T = singles.tile([128, 8, 64], F32)
        for k in range(8):
            tp = psum.tile([128, 64], F32, tag="tstage", bufs=3)
            nc.tensor.transpose(
                tp, w1_nat[0:64, 128 * k : 128 * (k + 1)], ident[0:64, 0:64]
            )
            nc.vector.tensor_copy(w1T[:, k, :], tp)

        # w2T: [k=64, m=C over 4 tiles] packed as [64, 4, 128]
        w2_nat = singles.tile([128, CB, 64], F32)
        for m in range(CB):
            nc.sync.dma_start(
                out=w2_nat[:, m, :], in_=w2_d[128 * m : 128 * (m + 1), :]
            )
        w2T = singles.tile([64, CB, 128], F32)
        for m in range(CB):
            tp = psum.tile([128, 128], F32, tag="tstage", bufs=3)
            nc.tensor.transpose(tp[0:64, :], w2_nat[:, m, :], ident)
            nc.vector.tensor_copy(w2T[:, m, :], tp[0:64, :])

        b1_t = singles.tile([64, 1], F32)
        nc.sync.dma_start(out=b1_t, in_=b1_d)
        b2_t = singles.tile([128, CB], F32)
        for m in range(CB):
            nc.sync.dma_start(out=b2_t[:, m : m + 1], in_=b2_d[128 * m : 128 * (m + 1), :])

        # ---- per-sample pipeline -------------------------------------------
        for rep in range(reps):
          for b in range(B_PER_CORE):
            # 1. q = x[b], chunked so compute starts as data streams in
            q = qpool.tile([128, CB, N], F32R, tag="q", name=f"q_s{rep}_{b}")
            for j in range(NCH // 2):
                nsl = slice(1024 * j, 1024 * (j + 1))
                for m in range(CB):
                    nc.sync.dma_start(
                        out=q[:, m, nsl],
                        in_=x_d[b, 128 * m : 128 * (m + 1), nsl].bitcast(F32R),
                    )

            # 2. pooled_x via ACT Copy+accum, emitted inside the phase-3
            # loop (below) at points where the needed chunks have landed, so
            # the in-order ACT queue never stalls on late DMA.
            px_mean = stats.tile([128, CB], F32, tag="px")
            px_part = stats.tile([128, CB, 2], F32, tag="pxp")

            # 3. energy = q @ q.T via on-the-fly PE transposes (fp32r matmul).
            # energy is symmetric: compute only the upper-triangular blocks
            # (row-block m covers cols >= 128m) and mirror the rest after.
            eps = [
                psum.tile([128, C - 128 * m], F32, tag="bank", bufs=5,
                          name=f"eps_{rep}_{b}_{m}")
                for m in range(CB)
            ]
            for kt in range(KT):
                tps = psum.tile([128, C], F32, tag="tstage", bufs=3)
                sl = slice(128 * kt, 128 * (kt + 1))
                for m in range(CB):
                    nc.tensor.transpose(
                        tps[:, 128 * m : 128 * (m + 1)].bitcast(F32R),
                        q[:, m, sl],
                        ident_r,
                    )
                qt = qtpool.tile([128, C], F32R, tag="qt")
                nc.vector.tensor_copy(qt, tps)
                for m in range(CB):
                    nc.tensor.matmul(
                        eps[m],
                        lhsT=qt[:, 128 * m : 128 * (m + 1)],
                        rhs=qt[:, 128 * m :],
                        start=(kt == 0),
                        stop=(kt == KT - 1),
                    )
                # staggered pooled-x pieces: piece (m, h) reads chunks
                # 4h..4h+3, which are resident well before kt 16h+12+m
                if 12 <= kt < 16:
                    m_, h_ = kt - 12, 0
                elif 27 <= kt < 31:
                    m_, h_ = kt - 27, 1
                else:
                    m_ = None
                if m_ is not None:
                    hsl = slice(2048 * h_, 2048 * (h_ + 1))
                    pxs = stats.tile([128, 2048], F32, tag="pxs", bufs=1)
                    nc.scalar.activation(
                        out=pxs,
                        in_=q[:, m_, hsl].bitcast(F32),
                        func=AF.Copy,
                        accum_out=px_part[:, m_, h_ : h_ + 1],
                    )

            px_raw = stats.tile([128, CB], F32, tag="pxr")
            nc.vector.tensor_reduce(out=px_raw, in_=px_part, axis=AX.X, op=ALU.add)
            nc.scalar.mul(px_mean, px_raw, 1.0 / N)

            # 3b+4+5 fused, fully per-block pipelined: evacuate row-block m,
            # mirror its lower blocks, reduce, exponentiate, transpose into
            # the GT staging banks -- so DVE/ACT/PE hand off block-by-block.
            en = gpool.tile([128, CB, C], F32, tag="en")
            nmin = stats.tile([128, CB], F32, tag="nmin")
            G = gpool.tile([128, CB, C], F32, tag="G")
            S = stats.tile([128, CB], F32, tag="S")
            gstage = [
                psum.tile([128, C], F32, tag="bank", bufs=5, name=f"gst_{rep}_{b}_{k}")
                for k in range(CB)
            ]
            for m in range(CB):
                nc.vector.tensor_copy(en[:, m, 128 * m :], eps[m])
                if m > 0:
                    tps = psum.tile([128, C], F32, tag="tstage", bufs=3)
                    for j in range(m):
                        # block (m, j) = block (j, m)^T
                        nc.tensor.transpose(
                            tps[:, 128 * j : 128 * (j + 1)],
                            en[:, j, 128 * m : 128 * (m + 1)],
                            ident,
                        )
                    nc.vector.tensor_copy(en[:, m, : 128 * m], tps[:, : 128 * m])
                nc.vector.tensor_reduce(
                    out=nmin[:, m : m + 1], in_=en[:, m, :], axis=AX.X, op=ALU.min
                )
                nc.scalar.activation(
                    out=G[:, m, :],
                    in_=en[:, m, :],
                    func=AF.Exp,
                    bias=nmin[:, m : m + 1],
                    scale=-1.0,
                    accum_out=S[:, m : m + 1],
                )
                for k in range(CB):
                    nc.tensor.transpose(
                        gstage[k][:, 128 * m : 128 * (m + 1)],
                        G[:, m, 128 * k : 128 * (k + 1)],
                        ident,
                    )
            recipS = stats.tile([128, CB], F32, tag="rS")
            nc.vector.reciprocal(recipS, S)
            GT = gtpool.tile([128, CB, C], F32R, tag="GT")
            for k in range(CB):
                nc.vector.tensor_copy(GT[:, k, :], gstage[k])

            # 6. pooled_out = (G @ px_mean) / S
            ps_po = psum.tile([128, CB], F32, tag="tstage", bufs=3)
            for m in range(CB):
                for k in range(CB):
                    nc.tensor.matmul(
                        ps_po[:, m : m + 1],
                        lhsT=GT[:, k, 128 * m : 128 * (m + 1)].bitcast(F32),
                        rhs=px_mean[:, k : k + 1],
                        start=(k == 0),
                        stop=(k == CB - 1),
                    )
            po_mean = stats.tile([128, CB], F32, tag="po")
            for m in range(CB):
                nc.scalar.activation(
                    po_mean[:, m : m + 1], ps_po[:, m : m + 1], AF.Copy,
                    scale=recipS[:, m : m + 1],
                )

            # 7. SE gate: h = relu(w1@pooled+b1); se = sigmoid(w2@h+b2)
            ps_h = psum.tile([64, 1], F32, tag="tstage", bufs=3)
            for k in range(8):
                rhs = px_mean[:, k : k + 1] if k < 4 else po_mean[:, k - 4 : k - 3]
                nc.tensor.matmul(
                    ps_h,
                    lhsT=w1T[:, k, :],
                    rhs=rhs,
                    start=(k == 0),
                    stop=(k == 7),
                )
            h_sb = stats.tile([64, 1], F32, tag="h")
            nc.scalar.activation(h_sb, ps_h, AF.Relu, bias=b1_t)

            ps_se = psum.tile([128, CB], F32, tag="tstage", bufs=3)
            for m in range(CB):
                nc.tensor.matmul(
                    ps_se[:, m : m + 1],
                    lhsT=w2T[:, m, :],
                    rhs=h_sb,
                    start=True,
                    stop=True,
                )
            se = stats.tile([128, CB], F32, tag="se")
            for m in range(CB):
                nc.scalar.activation(
                    se[:, m : m + 1], ps_se[:, m : m + 1], AF.Sigmoid,
                    bias=b2_t[:, m : m + 1],
                )
            beta0 = stats.tile([128, CB], F32, tag="b0")
            beta = stats.tile([128, CB], F32, tag="b1")
            nc.vector.tensor_scalar(
                out=beta0, in0=se, scalar1=-1.0, scalar2=1.0, op0=ALU.mult, op1=ALU.add
            )
            nc.vector.tensor_mul(beta, beta0, recipS)

            # 8. out_raw = G @ q with stationary reuse: for each (m, k) the
            # same lhsT serves all 8 n-chunks (walrus ldw-opt keeps the PE
            # weight buffer warm), accumulating into 8 live PSUM banks.
            for m in range(CB):
                for half in range(2):
                    j0 = 4 * half
                    banks = [
                        psum.tile([128, 512], F32, tag="bank", bufs=5,
                                  name=f"po_{rep}_{b}_{m}_{j0 + jj}")
                        for jj in range(4)
                    ]
                    for k in range(CB):
                        for jj in range(4):
                            j = j0 + jj
                            nc.tensor.matmul(
                                banks[jj],
                                lhsT=GT[:, k, 128 * m : 128 * (m + 1)],
                                rhs=q[:, k, 512 * j : 512 * (j + 1)],
                                start=(k == 0),
                                stop=(k == CB - 1),
                            )
                    for jp in range(2):
                        fin = outp.tile([128, 2, 512], F32, tag="fin", bufs=3)
                        for jj in range(2):
                            j = j0 + 2 * jp + jj
                            nsl = slice(512 * j, 512 * (j + 1))
                            ob = outp.tile([128, 512], F32, tag="ob", bufs=4)
                            nc.scalar.activation(
                                ob, banks[2 * jp + jj], AF.Copy,
                                scale=beta[:, m : m + 1],
                            )
                            nc.vector.scalar_tensor_tensor(
                                out=fin[:, jj, :],
                                in0=q[:, m, nsl].bitcast(F32),
                                scalar=se[:, m : m + 1],
                                in1=ob,
                                op0=ALU.mult,
                                op1=ALU.add,
                            )
                        nc.sync.dma_start(
                            out=y_d[b, 128 * m : 128 * (m + 1),
                                    1024 * (2 * half + jp) : 1024 * (2 * half + jp + 1)],
                            in_=fin,
                        )


_NC_CACHE = None


def _get_program():
    global _NC_CACHE
    if _NC_CACHE is None:
        _NC_CACHE = _build_program()
    return _NC_CACHE


def kernel(x, w1, b1, w2, b2, _trace=False):
    x = np.ascontiguousarray(x, dtype=np.float32)
    B, Cc, H, W = x.shape
    assert (B, Cc, H * W) == (B_TOTAL, C, N)
    xr = x.reshape(B, Cc, H * W)
    in_maps = []
    for i in range(N_CORES):
        in_maps.append(
            {
                "x": np.ascontiguousarray(xr[B_PER_CORE * i : B_PER_CORE * (i + 1)]),
                "w1": np.ascontiguousarray(w1, dtype=np.float32),
                "b1": np.ascontiguousarray(b1, dtype=np.float32).reshape(64, 1),
                "w2": np.ascontiguousarray(w2, dtype=np.float32),
                "b2": np.ascontiguousarray(b2, dtype=np.float32).reshape(C, 1),
            }
        )
    nc = _get_program()
    res = run_bass_kernel_spmd(nc, in_maps, list(range(N_CORES)), trace=_trace)
    y = np.concatenate([res.results[i]["y"] for i in range(N_CORES)], axis=0)
    out = y.reshape(B, Cc, H, W).astype(np.float32)
    if _trace:
        return out, res
    return out



# revision 2
# speedup vs baseline: 2.1429x; 2.1429x over previous
"""Trainium2 Bass kernel for nn_CA_Module (channel-attention + SE gating), v4.

Reference computation per sample (C=512, N=H*W=4096):
    q = x.reshape(C, N)
    energy = q @ q.T                     # [C, C]
    att = softmax(max_row - energy)      # == softmax(-energy)  (shift cancels)
        -> G = exp(min_row - energy); att = G / rowsum(G)
    out = att @ q                        # [C, N]
    pooled = concat([mean_n(x), mean_n(out)])        # [2C]
    h  = relu(w1 @ pooled + b1)                      # [64]
    se = sigmoid(w2 @ h + b2)                        # [C]
    y  = se * x + (1 - se) * out

Design (v4):
  * x is uploaded as fp16 (host converts); y is written fp16 and upconverted
    on host. Halves HBM traffic. fp16 (not bf16): the data is unit-scale
    gaussian, energies are |e| < 5000 << 65504, and fp16's 10-bit mantissa
    keeps the attention-gap perturbation ~8x smaller than bf16 at identical
    PE speed (1 cyc/row).
  * q^T tiles come straight from DRAM through the DMA crossbar transpose
    (batched: one [512,512] -> [128,4,512] instruction per 4 kt-slices), so
    the PE runs no transposes for mm1 and no PSUM staging copies exist.
  * G^T likewise: one [128,512] -> [128,4,128-col] crossbar transpose per
    row-block, issued on the ACT queue right behind the exp producing G.
  * the final blend folds into the attention matmul:
        y = se*x + beta*(G@q)  with beta=(1-se)/S  and x rows == q rows
          = beta * (G + diag(se/beta)) @ q
    so PSUM evacuation is one copy-with-scale per chunk (split ACT/DVE).
  * pooled_x via DVE tensor_scalar accumulate, staggered in the mm1 loop.
  * mm2 accumulates k==m last: only the diagonal-block term waits for `se`,
    and the first chunk's 12 off-diagonal matmuls hide the SE-gate chain.
  * y stores issue from the ACT queue (waits resolved by the preceding
    evacuations), x/qt from SP: no cross-stream head-of-line blocking.

Sharding: data-parallel over batch, 2 samples per core on 8 cores.

NOTE: runs with walrus's default --enable-ldw-opt=false: the tile scheduler
emits standalone Ldweights for 2-byte matmuls (same background weight-load
overlap at the IR level), and walrus rejects standalone Ldweights when
ldw-opt is on.
"""

import numpy as np

try:
    import concourse.bass as bass
except ImportError:
    import sys

    sys.path.insert(0, "/opt/trn_rl_repo")
    import concourse.bass as bass

import concourse.tile as tile
from concourse import bacc, mybir
from concourse.bass_utils import run_bass_kernel_spmd
from concourse.masks import make_identity

F32 = mybir.dt.float32
F16 = mybir.dt.float16
AF = mybir.ActivationFunctionType
ALU = mybir.AluOpType
AX = mybir.AxisListType

B_TOTAL = 16
N_CORES = 8
B_PER_CORE = B_TOTAL // N_CORES  # 2
C = 512
N = 4096
CB = C // 128  # 4 c-blocks
KT = N // 128  # 32 n-slices for mm1
JT = KT // 4   # 8 transpose groups (4 kt each)


def _build_program(reps: int = 1) -> bass.Bass:
    nc = bacc.Bacc(target_bir_lowering=False, debug=False)

    x_d = nc.dram_tensor("x", [B_PER_CORE, C, N], F16, kind="ExternalInput").ap()
    w1_d = nc.dram_tensor("w1", [64, 2 * C], F32, kind="ExternalInput").ap()
    b1_d = nc.dram_tensor("b1", [64, 1], F32, kind="ExternalInput").ap()
    w2_d = nc.dram_tensor("w2", [C, 64], F32, kind="ExternalInput").ap()
    b2_d = nc.dram_tensor("b2", [C, 1], F32, kind="ExternalInput").ap()
    y_d = nc.dram_tensor("y", [B_PER_CORE, C, N], F16, kind="ExternalOutput").ap()

    with tile.TileContext(nc) as tc:
        _emit(tc, x_d, w1_d, b1_d, w2_d, b2_d, y_d, reps)
    nc.compile()
    return nc


def _emit(tc, x_d, w1_d, b1_d, w2_d, b2_d, y_d, reps=1):
    nc = tc.nc
    from contextlib import ExitStack

    with ExitStack() as ctx:
        singles = ctx.enter_context(tc.tile_pool(name="singles", bufs=1))
        qpool = ctx.enter_context(tc.tile_pool(name="qpool", bufs=2))
        qtpool = ctx.enter_context(tc.tile_pool(name="qtpool", bufs=6))
        gpool = ctx.enter_context(tc.tile_pool(name="gpool", bufs=1))
        gtpool = ctx.enter_context(tc.tile_pool(name="gtpool", bufs=2))
        stats = ctx.enter_context(tc.tile_pool(name="stats", bufs=2))
        outp = ctx.enter_context(tc.tile_pool(name="outp", bufs=3))
        psum = ctx.enter_context(tc.tile_pool(name="psum", bufs=1, space="PSUM"))

        # ---- one-time setup -------------------------------------------------
        ident = singles.tile([128, 128], F32)
        make_identity(nc, ident)
        identh = singles.tile([128, 128], F16)
        nc.vector.tensor_copy(identh, ident)
        # warm-up transpose: absorb identity-producer waits into the PE clock
        warm = psum.tile([128, 128], F32, tag="tstage", bufs=2)
        nc.tensor.transpose(warm, ident, ident)

        # w1T: [k=2C partitions over 8 tiles, m=64] packed as [128, 8*64]
        w1_nat = singles.tile([64, 2 * C], F32)
        nc.sync.dma_start(out=w1_nat, in_=w1_d)
        w1T = singles.tile([128, 8, 64], F32)
        for k in range(8):
            tp = psum.tile([128, 64], F32, tag="tstage", bufs=2)
            nc.tensor.transpose(
                tp, w1_nat[0:64, 128 * k : 128 * (k + 1)], ident[0:64, 0:64]
            )
            nc.vector.tensor_copy(w1T[:, k, :], tp)

        # w2T: [k=64, m=C over 4 tiles] packed as [64, 4, 128]
        w2_nat = singles.tile([128, CB, 64], F32)
        for m in range(CB):
            nc.sync.dma_start(
                out=w2_nat[:, m, :], in_=w2_d[128 * m : 128 * (m + 1), :]
            )
        w2T = singles.tile([64, CB, 128], F32)
        for m in range(CB):
            tp = psum.tile([128, 128], F32, tag="tstage", bufs=2)
            nc.tensor.transpose(tp[0:64, :], w2_nat[:, m, :], ident)
            nc.vector.tensor_copy(w2T[:, m, :], tp[0:64, :])

        b1_t = singles.tile([64, 1], F32)
        nc.sync.dma_start(out=b1_t, in_=b1_d)
        b2_t = singles.tile([128, CB], F32)
        for m in range(CB):
            nc.sync.dma_start(out=b2_t[:, m : m + 1], in_=b2_d[128 * m : 128 * (m + 1), :])

        # one persistent tiny PSUM slot for the SE-gate matvecs:
        # cols 0-3 = pooled_out, col 4 = h (rows 0-63), cols 5-8 = se
        tiny = psum.tile([128, 12], F32, tag="tiny", bufs=1)

        # ---- per-sample pipeline -------------------------------------------
        for rep in range(reps):
          for b in range(B_PER_CORE):
            # 1a. q^T groups straight from DRAM via crossbar transpose:
            # qt4[:, jj, :] holds (x[:, 128*(4*j+jj):128*(4*j+jj+1)])^T
            qts = []
            for j in range(JT):
                qt4 = qtpool.tile([128, 4, C], F16, tag="qt",
                                  name=f"qt_{rep}_{b}_{j}")
                nc.sync.dma_start_transpose(
                    out=qt4, in_=x_d[b, :, 512 * j : 512 * (j + 1)]
                )
                qts.append(qt4)

            # 1b. q itself (mm2 moving operand + pooled-x source)
            q = qpool.tile([128, CB, N], F16, tag="q", name=f"q_s{rep}_{b}")
            for j in range(2):
                nsl = slice(2048 * j, 2048 * (j + 1))
                for m in range(CB):
                    nc.sync.dma_start(
                        out=q[:, m, nsl],
                        in_=x_d[b, 128 * m : 128 * (m + 1), nsl],
                    )

            # 2. pooled_x partials on DVE (tensor_scalar accum), staggered
            px_part = stats.tile([128, CB, 2], F32, tag="pxp")
            pxscr = stats.tile([128, 2048], F16, tag="pxscr", bufs=1)

            # 3. energy = q @ q.T (fp16, fp32 accumulate), upper-tri blocks
            eps = [
                psum.tile([128, C - 128 * m], F32, tag="bank", bufs=5,
                          name=f"eps_{rep}_{b}_{m}")
                for m in range(CB)
            ]
            for kt in range(KT):
                qt = qts[kt // 4][:, kt % 4, :]
                for m in range(CB):
                    nc.tensor.matmul(
                        eps[m],
                        lhsT=qt[:, 128 * m : 128 * (m + 1)],
                        rhs=qt[:, 128 * m :],
                        start=(kt == 0),
                        stop=(kt == KT - 1),
                    )
                if 12 <= kt < 16:
                    m_, h_ = kt - 12, 0
                elif 20 <= kt < 24:
                    m_, h_ = kt - 20, 1
                else:
                    m_ = None
                if m_ is not None:
                    hsl = slice(2048 * h_, 2048 * (h_ + 1))
                    nc.vector.tensor_scalar(
                        out=pxscr,
                        in0=q[:, m_, hsl],
                        scalar1=1.0,
                        scalar2=0.0,
                        op0=ALU.mult,
                        op1=ALU.add,
                        accum_out=px_part[:, m_, h_ : h_ + 1],
                    )

            px_mean = stats.tile([128, CB], F32, tag="px")
            px_raw = stats.tile([128, CB], F32, tag="pxr")
            nc.vector.tensor_reduce(out=px_raw, in_=px_part, axis=AX.X, op=ALU.add)
            nc.scalar.mul(px_mean, px_raw, 1.0 / N)
            px_h = stats.tile([128, CB], F16, tag="pxb")
            nc.vector.tensor_copy(px_h, px_mean)

            # 4+5 per-block pipeline: evacuate row-block m, mirror its lower
            # blocks, min, exp (fp16 G + fp32 rowsum), then G^T via one
            # crossbar transpose issued right behind the exp on the ACT queue.
            en = gpool.tile([128, CB, C], F32, tag="en")
            nmin = stats.tile([128, CB], F32, tag="nmin")
            G = gpool.tile([128, CB, C], F16, tag="G")
            S = stats.tile([128, CB], F32, tag="S")
            GT = gtpool.tile([128, CB, C], F16, tag="GT")
            gstage = [
                psum.tile([128, C], F16, tag="bank", bufs=5,
                          name=f"gst_{rep}_{b}_{k}")
                for k in range(CB)
            ]
            for m in range(CB):
                nc.vector.tensor_copy(en[:, m, 128 * m :], eps[m])
                if m > 0:
                    tps = psum.tile([128, C], F32, tag="tstage", bufs=2)
                    for j in range(m):
                        # block (m, j) = block (j, m)^T
                        nc.tensor.transpose(
                            tps[:, 128 * j : 128 * (j + 1)],
                            en[:, j, 128 * m : 128 * (m + 1)],
                            ident,
                        )
                    nc.vector.tensor_copy(en[:, m, : 128 * m], tps[:, : 128 * m])
                nc.vector.tensor_reduce(
                    out=nmin[:, m : m + 1], in_=en[:, m, :], axis=AX.X, op=ALU.min
                )
                nc.scalar.activation(
                    out=G[:, m, :],
                    in_=en[:, m, :],
                    func=AF.Exp,
                    bias=nmin[:, m : m + 1],
                    scale=-1.0,
                    accum_out=S[:, m : m + 1],
                )
                # (G row-block m)^T on the PE (off the DMA pipe, which
                # is busy prefetching the next sample's q^T tiles)
                for k in range(CB):
                    nc.tensor.transpose(
                        gstage[k][:, 128 * m : 128 * (m + 1)],
                        G[:, m, 128 * k : 128 * (k + 1)],
                        identh,
                    )
                for k in range(CB):
                    nc.vector.tensor_copy(
                        GT[:, k, 128 * m : 128 * (m + 1)],
                        gstage[k][:, 128 * m : 128 * (m + 1)],
                    )

            # 6. pooled_out = (G @ px_mean) / S
            for m in range(CB):
                for k in range(CB):
                    nc.tensor.matmul(
                        tiny[:, m : m + 1],
                        lhsT=GT[:, k, 128 * m : 128 * (m + 1)],
                        rhs=px_h[:, k : k + 1],
                        start=(k == 0),
                        stop=(k == CB - 1),
                    )
            recipS = stats.tile([128, CB], F32, tag="rS")
            nc.vector.reciprocal(recipS, S)
            po_mean = stats.tile([128, CB], F32, tag="po")
            for m in range(CB):
                nc.scalar.activation(
                    po_mean[:, m : m + 1], tiny[:, m : m + 1], AF.Copy,
                    scale=recipS[:, m : m + 1],
                )

            # 8a. first mm2 chunk's off-diagonal terms start now and hide the
            # whole SE-gate latency chain below.
            m0banks = [
                psum.tile([128, 512], F32, tag="bank", bufs=5,
                          name=f"po_{rep}_{b}_0_{jj}")
                for jj in range(4)
            ]
            for ki, k in enumerate((1, 2, 3)):
                for jj in range(4):
                    nc.tensor.matmul(
                        m0banks[jj],
                        lhsT=GT[:, k, 0:128],
                        rhs=q[:, k, 512 * jj : 512 * (jj + 1)],
                        start=(ki == 0),
                        stop=False,
                    )

            # 7. SE gate: h = relu(w1@pooled+b1); se = sigmoid(w2@h+b2)
            for k in range(8):
                rhs = px_mean[:, k : k + 1] if k < 4 else po_mean[:, k - 4 : k - 3]
                nc.tensor.matmul(
                    tiny[0:64, 4:5],
                    lhsT=w1T[:, k, :],
                    rhs=rhs,
                    start=(k == 0),
                    stop=(k == 7),
                )
            h_sb = stats.tile([64, 1], F32, tag="h")
            nc.scalar.activation(h_sb, tiny[0:64, 4:5], AF.Relu, bias=b1_t)

            for m in range(CB):
                nc.tensor.matmul(
                    tiny[:, 5 + m : 6 + m],
                    lhsT=w2T[:, m, :],
                    rhs=h_sb,
                    start=True,
                    stop=True,
                )
            se = stats.tile([128, CB], F32, tag="se")
            for m in range(CB):
                nc.scalar.activation(
                    se[:, m : m + 1], tiny[:, 5 + m : 6 + m], AF.Sigmoid,
                    bias=b2_t[:, m : m + 1],
                )
            # beta = (1-se)/S ; dvec = se/beta = se*S/(1-se)
            beta0 = stats.tile([128, CB], F32, tag="b0")
            beta = stats.tile([128, CB], F32, tag="b1")
            dvec = stats.tile([128, CB], F32, tag="dv")
            rb0 = stats.tile([128, CB], F32, tag="rb0")
            seS = stats.tile([128, CB], F32, tag="seS")
            nc.vector.tensor_scalar(
                out=beta0, in0=se, scalar1=-1.0, scalar2=1.0, op0=ALU.mult, op1=ALU.add
            )
            nc.vector.tensor_mul(beta, beta0, recipS)
            nc.vector.reciprocal(rb0, beta0)
            nc.vector.tensor_mul(seS, se, S)
            nc.vector.tensor_mul(dvec, seS, rb0)
            # fold diag(dvec) into GT's diagonal blocks:
            # GT[:, m, mcols] += ident * dvec[:, m]
            for m in range(CB):
                nc.vector.scalar_tensor_tensor(
                    out=GT[:, m, 128 * m : 128 * (m + 1)],
                    in0=identh,
                    scalar=dvec[:, m : m + 1],
                    in1=GT[:, m, 128 * m : 128 * (m + 1)],
                    op0=ALU.mult,
                    op1=ALU.add,
                )

            # 8b. y_raw = (G + diag(se/beta)) @ q, k==m last; evacuate with
            # the beta scale split across ACT and DVE, store fp16 from the
            # ACT queue in 2KB/partition chunks.
            def mm2_evac(m, half, banks):
                fin = outp.tile([128, 4, 512], F16, tag="fin", bufs=3)
                for jj in range(4):
                    if jj % 2 == 0:
                        nc.scalar.activation(
                            fin[:, jj, :], banks[jj], AF.Copy,
                            scale=beta[:, m : m + 1],
                        )
                    else:
                        nc.vector.tensor_scalar(
                            out=fin[:, jj, :], in0=banks[jj],
                            scalar1=beta[:, m : m + 1], scalar2=None,
                            op0=ALU.mult,
                        )
                nc.scalar.dma_start(
                    out=y_d[b, 128 * m : 128 * (m + 1),
                            2048 * half : 2048 * (half + 1)],
                    in_=fin,
                )

            # finish m=0, half=0 (diagonal term), then evacuate
            for jj in range(4):
                nc.tensor.matmul(
                    m0banks[jj],
                    lhsT=GT[:, 0, 0:128],
                    rhs=q[:, 0, 512 * jj : 512 * (jj + 1)],
                    start=False,
                    stop=True,
                )
            mm2_evac(0, 0, m0banks)

            for m in range(CB):
                korder = [k for k in range(CB) if k != m] + [m]
                for half in range(2):
                    if m == 0 and half == 0:
                        continue
                    j0 = 4 * half
                    banks = [
                        psum.tile([128, 512], F32, tag="bank", bufs=5,
                                  name=f"po_{rep}_{b}_{m}_{j0 + jj}")
                        for jj in range(4)
                    ]
                    for ki, k in enumerate(korder):
                        for jj in range(4):
                            j = j0 + jj
                            nc.tensor.matmul(
                                banks[jj],
                                lhsT=GT[:, k, 128 * m : 128 * (m + 1)],
                                rhs=q[:, k, 512 * j : 512 * (j + 1)],
                                start=(ki == 0),
                                stop=(ki == CB - 1),
                            )
                    mm2_evac(m, half, banks)


_NC_CACHE = None


def _get_program():
    global _NC_CACHE
    if _NC_CACHE is None:
        _NC_CACHE = _build_program()
    return _NC_CACHE


def kernel(x, w1, b1, w2, b2, _trace=False):
    x = np.ascontiguousarray(x, dtype=np.float32)
    B, Cc, H, W = x.shape
    assert (B, Cc, H * W) == (B_TOTAL, C, N)
    xr = x.reshape(B, Cc, H * W).astype(np.float16)
    in_maps = []
    for i in range(N_CORES):
        in_maps.append(
            {
                "x": np.ascontiguousarray(xr[B_PER_CORE * i : B_PER_CORE * (i + 1)]),
                "w1": np.ascontiguousarray(w1, dtype=np.float32),
                "b1": np.ascontiguousarray(b1, dtype=np.float32).reshape(64, 1),
                "w2": np.ascontiguousarray(w2, dtype=np.float32),
                "b2": np.ascontiguousarray(b2, dtype=np.float32).reshape(C, 1),
            }
        )
    nc = _get_program()
    res = run_bass_kernel_spmd(nc, in_maps, list(range(N_CORES)), trace=_trace)
    y = np.concatenate([res.results[i]["y"] for i in range(N_CORES)], axis=0)
    out = y.reshape(B, Cc, H, W).astype(np.float32)
    if _trace:
        return out, res
    return out


# revision 3
# speedup vs baseline: 2.4545x; 1.1455x over previous
"""Trainium2 Bass kernel for nn_CA_Module (channel-attention + SE gating), v4.

Reference computation per sample (C=512, N=H*W=4096):
    q = x.reshape(C, N)
    energy = q @ q.T                     # [C, C]
    att = softmax(max_row - energy)      # == softmax(-energy)  (shift cancels)
        -> G = exp(min_row - energy); att = G / rowsum(G)
    out = att @ q                        # [C, N]
    pooled = concat([mean_n(x), mean_n(out)])        # [2C]
    h  = relu(w1 @ pooled + b1)                      # [64]
    se = sigmoid(w2 @ h + b2)                        # [C]
    y  = se * x + (1 - se) * out

Design (v4):
  * x is uploaded as fp16 (host converts); y is written fp16 and upconverted
    on host. Halves HBM traffic. fp16 (not bf16): the data is unit-scale
    gaussian, energies are |e| < 5000 << 65504, and fp16's 10-bit mantissa
    keeps the attention-gap perturbation ~8x smaller than bf16 at identical
    PE speed (1 cyc/row).
  * q^T tiles come straight from DRAM through the DMA crossbar transpose
    (batched: one [512,512] -> [128,4,512] instruction per 4 kt-slices), so
    the PE runs no transposes for mm1 and no PSUM staging copies exist.
  * G^T likewise: one [128,512] -> [128,4,128-col] crossbar transpose per
    row-block, issued on the ACT queue right behind the exp producing G.
  * the final blend folds into the attention matmul:
        y = se*x + beta*(G@q)  with beta=(1-se)/S  and x rows == q rows
          = beta * (G + diag(se/beta)) @ q
    so PSUM evacuation is one copy-with-scale per chunk (split ACT/DVE).
  * pooled_x via DVE tensor_scalar accumulate, staggered in the mm1 loop.
  * mm2 accumulates k==m last: only the diagonal-block term waits for `se`,
    and the first chunk's 12 off-diagonal matmuls hide the SE-gate chain.
  * y stores issue from the ACT queue (waits resolved by the preceding
    evacuations), x/qt from SP: no cross-stream head-of-line blocking.

Sharding: data-parallel over batch, 2 samples per core on 8 cores.

NOTE: runs with walrus's default --enable-ldw-opt=false: the tile scheduler
emits standalone Ldweights for 2-byte matmuls (same background weight-load
overlap at the IR level), and walrus rejects standalone Ldweights when
ldw-opt is on.
"""

import numpy as np

try:
    import concourse.bass as bass
except ImportError:
    import sys

    sys.path.insert(0, "/opt/trn_rl_repo")
    import concourse.bass as bass

import concourse.tile as tile
from concourse import bacc, mybir
from concourse.bass_utils import run_bass_kernel_spmd
from concourse.masks import make_identity

F32 = mybir.dt.float32
F16 = mybir.dt.float16
AF = mybir.ActivationFunctionType
ALU = mybir.AluOpType
AX = mybir.AxisListType

B_TOTAL = 16
N_CORES = 8
B_PER_CORE = B_TOTAL // N_CORES  # 2
C = 512
N = 4096
CB = C // 128  # 4 c-blocks
KT = N // 128  # 32 n-slices for mm1
JT = KT // 4   # 8 transpose groups (4 kt each)


def _build_program(reps: int = 1) -> bass.Bass:
    nc = bacc.Bacc(target_bir_lowering=False, debug=False)

    x_d = nc.dram_tensor("x", [B_PER_CORE, C, N], F16, kind="ExternalInput").ap()
    w1_d = nc.dram_tensor("w1", [64, 2 * C], F32, kind="ExternalInput").ap()
    b1_d = nc.dram_tensor("b1", [64, 1], F32, kind="ExternalInput").ap()
    w2_d = nc.dram_tensor("w2", [C, 64], F32, kind="ExternalInput").ap()
    b2_d = nc.dram_tensor("b2", [C, 1], F32, kind="ExternalInput").ap()
    y_d = nc.dram_tensor("y", [B_PER_CORE, C, N], F16, kind="ExternalOutput").ap()

    with tile.TileContext(nc) as tc:
        _emit(tc, x_d, w1_d, b1_d, w2_d, b2_d, y_d, reps)
    nc.compile()
    return nc


def _emit(tc, x_d, w1_d, b1_d, w2_d, b2_d, y_d, reps=1):
    nc = tc.nc
    from contextlib import ExitStack

    with ExitStack() as ctx:
        singles = ctx.enter_context(tc.tile_pool(name="singles", bufs=1))
        qpool = ctx.enter_context(tc.tile_pool(name="qpool", bufs=2))
        qtpool = ctx.enter_context(tc.tile_pool(name="qtpool", bufs=8))
        gpool = ctx.enter_context(tc.tile_pool(name="gpool", bufs=1))
        gtpool = ctx.enter_context(tc.tile_pool(name="gtpool", bufs=2))
        stats = ctx.enter_context(tc.tile_pool(name="stats", bufs=2))
        outp = ctx.enter_context(tc.tile_pool(name="outp", bufs=3))
        psum = ctx.enter_context(tc.tile_pool(name="psum", bufs=1, space="PSUM"))

        # ---- prefetch the first sample's inputs -----------------------------
        # issued before anything else so the crossbar transposes and x loads
        # own the DMA pipe from t=0; the weight-prep below overlaps them
        pre_qts = []
        for j in range(JT):
            qt4 = qtpool.tile([128, 4, C], F16, tag="qt", name=f"qt_pre_{j}")
            nc.sync.dma_start_transpose(
                out=qt4, in_=x_d[0, :, 512 * j : 512 * (j + 1)]
            )
            pre_qts.append(qt4)
        pre_q = qpool.tile([128, CB, N], F16, tag="q", name="q_pre")
        for j in range(2):
            nsl = slice(2048 * j, 2048 * (j + 1))
            for m in range(CB):
                nc.sync.dma_start(
                    out=pre_q[:, m, nsl],
                    in_=x_d[0, 128 * m : 128 * (m + 1), nsl],
                )

        # ---- one-time setup -------------------------------------------------
        ident = singles.tile([128, 128], F32)
        make_identity(nc, ident)
        identh = singles.tile([128, 128], F16)
        nc.vector.tensor_copy(identh, ident)
        # warm-up transpose: absorb identity-producer waits into the PE clock
        warm = psum.tile([128, 128], F32, tag="tstage", bufs=2)
        nc.tensor.transpose(warm, ident, ident)

        # w1T: [k=2C partitions over 8 tiles, m=64] packed as [128, 8*64]
        w1_nat = singles.tile([64, 2 * C], F32)
        nc.sync.dma_start(out=w1_nat, in_=w1_d)
        w1T = singles.tile([128, 8, 64], F32)
        for k in range(8):
            tp = psum.tile([128, 64], F32, tag="tstage", bufs=2)
            nc.tensor.transpose(
                tp, w1_nat[0:64, 128 * k : 128 * (k + 1)], ident[0:64, 0:64]
            )
            nc.vector.tensor_copy(w1T[:, k, :], tp)

        # w2T: [k=64, m=C over 4 tiles] packed as [64, 4, 128]
        w2_nat = singles.tile([128, CB, 64], F32)
        for m in range(CB):
            nc.sync.dma_start(
                out=w2_nat[:, m, :], in_=w2_d[128 * m : 128 * (m + 1), :]
            )
        w2T = singles.tile([64, CB, 128], F32)
        for m in range(CB):
            tp = psum.tile([128, 128], F32, tag="tstage", bufs=2)
            nc.tensor.transpose(tp[0:64, :], w2_nat[:, m, :], ident)
            nc.vector.tensor_copy(w2T[:, m, :], tp[0:64, :])

        b1_t = singles.tile([64, 1], F32)
        nc.sync.dma_start(out=b1_t, in_=b1_d)
        b2_t = singles.tile([128, CB], F32)
        for m in range(CB):
            nc.sync.dma_start(out=b2_t[:, m : m + 1], in_=b2_d[128 * m : 128 * (m + 1), :])
        nb2_t = singles.tile([128, CB], F32)
        nc.vector.tensor_scalar(
            out=nb2_t, in0=b2_t, scalar1=-1.0, scalar2=None, op0=ALU.mult
        )

        # one persistent tiny PSUM slot for the SE-gate matvecs:
        # cols 0-3 = pooled_out, col 4 = h (rows 0-63), cols 5-8 = se
        tiny = psum.tile([128, 12], F32, tag="tiny", bufs=1)

        # ---- per-sample pipeline -------------------------------------------
        for rep in range(reps):
          for b in range(B_PER_CORE):
            # 1a/1b. q^T groups + q (prefetched before setup for the very
            # first sample; see top of _emit)
            if rep == 0 and b == 0:
                qts, q = pre_qts, pre_q
            else:
                qts = []
                for j in range(JT):
                    qt4 = qtpool.tile([128, 4, C], F16, tag="qt",
                                      name=f"qt_{rep}_{b}_{j}")
                    nc.sync.dma_start_transpose(
                        out=qt4, in_=x_d[b, :, 512 * j : 512 * (j + 1)]
                    )
                    qts.append(qt4)
                q = qpool.tile([128, CB, N], F16, tag="q", name=f"q_s{rep}_{b}")
                for j in range(2):
                    nsl = slice(2048 * j, 2048 * (j + 1))
                    for m in range(CB):
                        nc.sync.dma_start(
                            out=q[:, m, nsl],
                            in_=x_d[b, 128 * m : 128 * (m + 1), nsl],
                        )

            # 2. pooled_x partials on DVE (tensor_scalar accum), staggered
            px_part = stats.tile([128, CB, 2], F32, tag="pxp")
            pxscr = stats.tile([128, 2048], F16, tag="pxscr", bufs=1)

            # 3. energy = q @ q.T (fp16, fp32 accumulate), upper-tri blocks
            eps = [
                psum.tile([128, C - 128 * m], F32, tag="bank", bufs=5,
                          name=f"eps_{rep}_{b}_{m}")
                for m in range(CB)
            ]
            for kt in range(KT):
                qt = qts[kt // 4][:, kt % 4, :]
                for m in range(CB):
                    nc.tensor.matmul(
                        eps[m],
                        lhsT=qt[:, 128 * m : 128 * (m + 1)],
                        rhs=qt[:, 128 * m :],
                        start=(kt == 0),
                        stop=(kt == KT - 1),
                    )
                if 12 <= kt < 16:
                    m_, h_ = kt - 12, 0
                elif 16 <= kt < 20:
                    m_, h_ = kt - 16, 1
                else:
                    m_ = None
                if m_ is not None:
                    hsl = slice(2048 * h_, 2048 * (h_ + 1))
                    nc.vector.tensor_scalar(
                        out=pxscr,
                        in0=q[:, m_, hsl],
                        scalar1=1.0,
                        scalar2=0.0,
                        op0=ALU.mult,
                        op1=ALU.add,
                        accum_out=px_part[:, m_, h_ : h_ + 1],
                    )

            px_mean = stats.tile([128, CB], F32, tag="px")
            px_raw = stats.tile([128, CB], F32, tag="pxr")
            nc.vector.tensor_reduce(out=px_raw, in_=px_part, axis=AX.X, op=ALU.add)
            nc.scalar.mul(px_mean, px_raw, 1.0 / N)
            px_h = stats.tile([128, CB], F16, tag="pxb")
            nc.vector.tensor_copy(px_h, px_mean)

            # 4+5 per-block pipeline: evacuate row-block m, mirror its lower
            # blocks, min, exp (fp16 G + fp32 rowsum), then G^T via one
            # crossbar transpose issued right behind the exp on the ACT queue.
            en = gpool.tile([128, CB, C], F32, tag="en")
            nmin = stats.tile([128, CB], F32, tag="nmin")
            G = gpool.tile([128, CB, C], F16, tag="G")
            S = stats.tile([128, CB], F32, tag="S")
            GT = gtpool.tile([128, CB, C], F16, tag="GT")
            gstage = [
                psum.tile([128, C], F16, tag="bank", bufs=5,
                          name=f"gst_{rep}_{b}_{k}")
                for k in range(CB)
            ]
            for m in range(CB):
                # mirrors first: they read row-blocks j<m (already in SBUF),
                # so the PE transposes overlap this block's DVE evacuation
                if m > 0:
                    tps = psum.tile([128, C], F32, tag="tstage", bufs=2)
                    for j in range(m):
                        # block (m, j) = block (j, m)^T
                        nc.tensor.transpose(
                            tps[:, 128 * j : 128 * (j + 1)],
                            en[:, j, 128 * m : 128 * (m + 1)],
                            ident,
                        )
                nc.vector.tensor_copy(en[:, m, 128 * m :], eps[m])
                if m > 0:
                    nc.vector.tensor_copy(en[:, m, : 128 * m], tps[:, : 128 * m])
                nc.vector.tensor_reduce(
                    out=nmin[:, m : m + 1], in_=en[:, m, :], axis=AX.X, op=ALU.min
                )
                nc.scalar.activation(
                    out=G[:, m, :],
                    in_=en[:, m, :],
                    func=AF.Exp,
                    bias=nmin[:, m : m + 1],
                    scale=-1.0,
                    accum_out=S[:, m : m + 1],
                )
                # (G row-block m)^T on the PE (off the DMA pipe, which
                # is busy prefetching the next sample's q^T tiles)
                for k in range(CB):
                    nc.tensor.transpose(
                        gstage[k][:, 128 * m : 128 * (m + 1)],
                        G[:, m, 128 * k : 128 * (k + 1)],
                        identh,
                    )
                for k in range(CB):
                    nc.vector.tensor_copy(
                        GT[:, k, 128 * m : 128 * (m + 1)],
                        gstage[k][:, 128 * m : 128 * (m + 1)],
                    )

            # 6. pooled_out = (G @ px_mean) / S
            for m in range(CB):
                for k in range(CB):
                    nc.tensor.matmul(
                        tiny[:, m : m + 1],
                        lhsT=GT[:, k, 128 * m : 128 * (m + 1)],
                        rhs=px_h[:, k : k + 1],
                        start=(k == 0),
                        stop=(k == CB - 1),
                    )
            recipS = stats.tile([128, CB], F32, tag="rS")
            nc.vector.reciprocal(recipS, S)
            po_mean = stats.tile([128, CB], F32, tag="po")
            for m in range(CB):
                nc.scalar.activation(
                    po_mean[:, m : m + 1], tiny[:, m : m + 1], AF.Copy,
                    scale=recipS[:, m : m + 1],
                )

            # 8a. first mm2 chunk's off-diagonal terms start now and hide the
            # whole SE-gate latency chain below.
            m0banks = [
                psum.tile([128, 512], F32, tag="bank", bufs=5,
                          name=f"po_{rep}_{b}_0_{jj}")
                for jj in range(4)
            ]
            for ki, k in enumerate((1, 2, 3)):
                for jj in range(4):
                    nc.tensor.matmul(
                        m0banks[jj],
                        lhsT=GT[:, k, 0:128],
                        rhs=q[:, k, 512 * jj : 512 * (jj + 1)],
                        start=(ki == 0),
                        stop=False,
                    )

            # 7. SE gate: h = relu(w1@pooled+b1); se = sigmoid(w2@h+b2)
            for k in range(8):
                rhs = px_mean[:, k : k + 1] if k < 4 else po_mean[:, k - 4 : k - 3]
                nc.tensor.matmul(
                    tiny[0:64, 4:5],
                    lhsT=w1T[:, k, :],
                    rhs=rhs,
                    start=(k == 0),
                    stop=(k == 7),
                )
            h_sb = stats.tile([64, 1], F32, tag="h")
            nc.scalar.activation(h_sb, tiny[0:64, 4:5], AF.Relu, bias=b1_t)

            for m in range(CB):
                nc.tensor.matmul(
                    tiny[:, 5 + m : 6 + m],
                    lhsT=w2T[:, m, :],
                    rhs=h_sb,
                    start=True,
                    stop=True,
                )
            # sigmoid via exp so the whole kernel stays in one ACT
            # function-table set (exp_and_others): se = 1/(1+exp(-(z+b2)))
            se_e = stats.tile([128, CB], F32, tag="see")
            se_1 = stats.tile([128, CB], F32, tag="se1")
            se = stats.tile([128, CB], F32, tag="se")
            for m in range(CB):
                nc.scalar.activation(
                    se_e[:, m : m + 1], tiny[:, 5 + m : 6 + m], AF.Exp,
                    bias=nb2_t[:, m : m + 1], scale=-1.0,
                )
            nc.vector.tensor_scalar(
                out=se_1, in0=se_e, scalar1=1.0, scalar2=None, op0=ALU.add
            )
            nc.vector.reciprocal(se, se_1)
            # beta = (1-se)/S ; dvec = se/beta = se*S/(1-se)
            beta0 = stats.tile([128, CB], F32, tag="b0")
            beta = stats.tile([128, CB], F32, tag="b1")
            dvec = stats.tile([128, CB], F32, tag="dv")
            rb0 = stats.tile([128, CB], F32, tag="rb0")
            seS = stats.tile([128, CB], F32, tag="seS")
            nc.vector.tensor_scalar(
                out=beta0, in0=se, scalar1=-1.0, scalar2=1.0, op0=ALU.mult, op1=ALU.add
            )
            nc.vector.tensor_mul(beta, beta0, recipS)
            nc.vector.reciprocal(rb0, beta0)
            nc.vector.tensor_mul(seS, se, S)
            nc.vector.tensor_mul(dvec, seS, rb0)
            # fold diag(dvec) into GT's diagonal blocks:
            # GT[:, m, mcols] += ident * dvec[:, m]
            for m in range(CB):
                nc.vector.scalar_tensor_tensor(
                    out=GT[:, m, 128 * m : 128 * (m + 1)],
                    in0=identh,
                    scalar=dvec[:, m : m + 1],
                    in1=GT[:, m, 128 * m : 128 * (m + 1)],
                    op0=ALU.mult,
                    op1=ALU.add,
                )

            # 8b. y_raw = (G + diag(se/beta)) @ q, k==m last; evacuate with
            # the beta scale split across ACT and DVE, store fp16 from the
            # ACT queue in 2KB/partition chunks.
            def mm2_evac(m, half, banks, split=False):
                fin = outp.tile([128, 4, 512], F16, tag="fin", bufs=5)
                for jj in range(4):
                    if jj % 2 == 0:
                        nc.scalar.activation(
                            fin[:, jj, :], banks[jj], AF.Copy,
                            scale=beta[:, m : m + 1],
                        )
                    else:
                        nc.vector.tensor_scalar(
                            out=fin[:, jj, :], in0=banks[jj],
                            scalar1=beta[:, m : m + 1], scalar2=None,
                            op0=ALU.mult,
                        )
                    if split and jj % 2 == 1:
                        nc.sync.dma_start(
                            out=y_d[b, 128 * m : 128 * (m + 1),
                                    2048 * half + 1024 * (jj // 2) :
                                    2048 * half + 1024 * (jj // 2 + 1)],
                            in_=fin[:, jj - 1 : jj + 1, :],
                        )
                if not split:
                    nc.sync.dma_start(
                        out=y_d[b, 128 * m : 128 * (m + 1),
                                2048 * half : 2048 * (half + 1)],
                        in_=fin,
                    )

            # finish m=0, half=0 (diagonal term), then evacuate
            for jj in range(4):
                nc.tensor.matmul(
                    m0banks[jj],
                    lhsT=GT[:, 0, 0:128],
                    rhs=q[:, 0, 512 * jj : 512 * (jj + 1)],
                    start=False,
                    stop=True,
                )
            mm2_evac(0, 0, m0banks)

            for m in range(CB):
                korder = [k for k in range(CB) if k != m] + [m]
                for half in range(2):
                    if m == 0 and half == 0:
                        continue
                    j0 = 4 * half
                    banks = [
                        psum.tile([128, 512], F32, tag="bank", bufs=5,
                                  name=f"po_{rep}_{b}_{m}_{j0 + jj}")
                        for jj in range(4)
                    ]
                    for ki, k in enumerate(korder):
                        for jj in range(4):
                            j = j0 + jj
                            nc.tensor.matmul(
                                banks[jj],
                                lhsT=GT[:, k, 128 * m : 128 * (m + 1)],
                                rhs=q[:, k, 512 * j : 512 * (j + 1)],
                                start=(ki == 0),
                                stop=(ki == CB - 1),
                            )
                    mm2_evac(m, half, banks, split=(m == CB - 1 and half == 1))


_NC_CACHE = None


def _get_program():
    global _NC_CACHE
    if _NC_CACHE is None:
        _NC_CACHE = _build_program()
    return _NC_CACHE


def kernel(x, w1, b1, w2, b2, _trace=False):
    x = np.ascontiguousarray(x, dtype=np.float32)
    B, Cc, H, W = x.shape
    assert (B, Cc, H * W) == (B_TOTAL, C, N)
    xr = x.reshape(B, Cc, H * W).astype(np.float16)
    in_maps = []
    for i in range(N_CORES):
        in_maps.append(
            {
                "x": np.ascontiguousarray(xr[B_PER_CORE * i : B_PER_CORE * (i + 1)]),
                "w1": np.ascontiguousarray(w1, dtype=np.float32),
                "b1": np.ascontiguousarray(b1, dtype=np.float32).reshape(64, 1),
                "w2": np.ascontiguousarray(w2, dtype=np.float32),
                "b2": np.ascontiguousarray(b2, dtype=np.float32).reshape(C, 1),
            }
        )
    nc = _get_program()
    res = run_bass_kernel_spmd(nc, in_maps, list(range(N_CORES)), trace=_trace)
    y = np.concatenate([res.results[i]["y"] for i in range(N_CORES)], axis=0)
    out = y.reshape(B, Cc, H, W).astype(np.float32)
    if _trace:
        return out, res
    return out


# revision 4
# speedup vs baseline: 2.7000x; 1.1000x over previous
"""Trainium2 Bass kernel for nn_CA_Module (channel-attention + SE gating), v4.

Reference computation per sample (C=512, N=H*W=4096):
    q = x.reshape(C, N)
    energy = q @ q.T                     # [C, C]
    att = softmax(max_row - energy)      # == softmax(-energy)  (shift cancels)
        -> G = exp(min_row - energy); att = G / rowsum(G)
    out = att @ q                        # [C, N]
    pooled = concat([mean_n(x), mean_n(out)])        # [2C]
    h  = relu(w1 @ pooled + b1)                      # [64]
    se = sigmoid(w2 @ h + b2)                        # [C]
    y  = se * x + (1 - se) * out

Design (v4):
  * x is uploaded as fp16 (host converts); y is written fp16 and upconverted
    on host. Halves HBM traffic. fp16 (not bf16): the data is unit-scale
    gaussian, energies are |e| < 5000 << 65504, and fp16's 10-bit mantissa
    keeps the attention-gap perturbation ~8x smaller than bf16 at identical
    PE speed (1 cyc/row).
  * q^T tiles come straight from DRAM through the DMA crossbar transpose
    (batched: one [512,512] -> [128,4,512] instruction per 4 kt-slices), so
    the PE runs no transposes for mm1 and no PSUM staging copies exist.
  * G^T likewise: one [128,512] -> [128,4,128-col] crossbar transpose per
    row-block, issued on the ACT queue right behind the exp producing G.
  * the final blend folds into the attention matmul:
        y = se*x + beta*(G@q)  with beta=(1-se)/S  and x rows == q rows
          = beta * (G + diag(se/beta)) @ q
    so PSUM evacuation is one copy-with-scale per chunk (split ACT/DVE).
  * pooled_x via DVE tensor_scalar accumulate, staggered in the mm1 loop.
  * mm2 accumulates k==m last: only the diagonal-block term waits for `se`,
    and the first chunk's 12 off-diagonal matmuls hide the SE-gate chain.
  * y stores issue from the ACT queue (waits resolved by the preceding
    evacuations), x/qt from SP: no cross-stream head-of-line blocking.

Sharding: data-parallel over batch, 2 samples per core on 8 cores.

NOTE: runs with walrus's default --enable-ldw-opt=false: the tile scheduler
emits standalone Ldweights for 2-byte matmuls (same background weight-load
overlap at the IR level), and walrus rejects standalone Ldweights when
ldw-opt is on.
"""

import numpy as np

try:
    import concourse.bass as bass
except ImportError:
    import sys

    sys.path.insert(0, "/opt/trn_rl_repo")
    import concourse.bass as bass

import concourse.tile as tile
from concourse import bacc, mybir
from concourse.bass_utils import run_bass_kernel_spmd
from concourse.masks import make_identity

F32 = mybir.dt.float32
F16 = mybir.dt.float16
AF = mybir.ActivationFunctionType
ALU = mybir.AluOpType
AX = mybir.AxisListType

B_TOTAL = 16
N_CORES = 8
B_PER_CORE = B_TOTAL // N_CORES  # 2
C = 512
N = 4096
CB = C // 128  # 4 c-blocks
KT = N // 128  # 32 n-slices for mm1
JT = KT // 4   # 8 transpose groups (4 kt each)


def _build_program(reps: int = 1) -> bass.Bass:
    nc = bacc.Bacc(target_bir_lowering=False, debug=False)

    x_d = nc.dram_tensor("x", [B_PER_CORE, C, N], F16, kind="ExternalInput").ap()
    w1_d = nc.dram_tensor("w1", [64, 2 * C], F32, kind="ExternalInput").ap()
    b1_d = nc.dram_tensor("b1", [64, 1], F32, kind="ExternalInput").ap()
    w2_d = nc.dram_tensor("w2", [C, 64], F32, kind="ExternalInput").ap()
    b2_d = nc.dram_tensor("b2", [C, 1], F32, kind="ExternalInput").ap()
    y_d = nc.dram_tensor("y", [B_PER_CORE, C, N], F16, kind="ExternalOutput").ap()

    with tile.TileContext(nc) as tc:
        _emit(tc, x_d, w1_d, b1_d, w2_d, b2_d, y_d, reps)
    nc.compile()
    return nc


def _emit(tc, x_d, w1_d, b1_d, w2_d, b2_d, y_d, reps=1):
    nc = tc.nc
    from contextlib import ExitStack

    with ExitStack() as ctx:
        singles = ctx.enter_context(tc.tile_pool(name="singles", bufs=1))
        qpool = ctx.enter_context(tc.tile_pool(name="qpool", bufs=2))
        qtpool = ctx.enter_context(tc.tile_pool(name="qtpool", bufs=8))
        gpool = ctx.enter_context(tc.tile_pool(name="gpool", bufs=1))
        gtpool = ctx.enter_context(tc.tile_pool(name="gtpool", bufs=2))
        stats = ctx.enter_context(tc.tile_pool(name="stats", bufs=2))
        outp = ctx.enter_context(tc.tile_pool(name="outp", bufs=3))
        psum = ctx.enter_context(tc.tile_pool(name="psum", bufs=1, space="PSUM"))

        # ---- prefetch the first sample's inputs -----------------------------
        # issued before anything else so the crossbar transposes and x loads
        # own the DMA pipe from t=0; the weight-prep below overlaps them
        pre_qts = []
        for j in range(JT):
            qt4 = qtpool.tile([128, 4, C], F16, tag="qt", name=f"qt_pre_{j}")
            nc.sync.dma_start_transpose(
                out=qt4, in_=x_d[0, :, 512 * j : 512 * (j + 1)]
            )
            pre_qts.append(qt4)
        pre_q = qpool.tile([128, CB, N], F16, tag="q", name="q_pre")
        for m in range(CB):
            nc.sync.dma_start(
                out=pre_q[:, m, :],
                in_=x_d[0, 128 * m : 128 * (m + 1), :],
            )

        # ---- one-time setup -------------------------------------------------
        ident = singles.tile([128, 128], F32)
        make_identity(nc, ident)
        identh = singles.tile([128, 128], F16)
        nc.vector.tensor_copy(identh, ident)
        # warm-up transpose: absorb identity-producer waits into the PE clock
        warm = psum.tile([128, 128], F32, tag="tstage", bufs=2)
        nc.tensor.transpose(warm, ident, ident)

        # w1T: [k=2C partitions over 8 tiles, m=64] packed as [128, 8*64]
        w1_nat = singles.tile([64, 2 * C], F32)
        nc.sync.dma_start(out=w1_nat, in_=w1_d)
        w1T = singles.tile([128, 8, 64], F32)
        for k in range(8):
            tp = psum.tile([128, 64], F32, tag="tstage", bufs=2)
            nc.tensor.transpose(
                tp, w1_nat[0:64, 128 * k : 128 * (k + 1)], ident[0:64, 0:64]
            )
            nc.vector.tensor_copy(w1T[:, k, :], tp)

        # w2T: [k=64, m=C over 4 tiles] packed as [64, 4, 128]
        w2_nat = singles.tile([128, CB, 64], F32)
        for m in range(CB):
            nc.sync.dma_start(
                out=w2_nat[:, m, :], in_=w2_d[128 * m : 128 * (m + 1), :]
            )
        w2T = singles.tile([64, CB, 128], F32)
        for m in range(CB):
            tp = psum.tile([128, 128], F32, tag="tstage", bufs=2)
            nc.tensor.transpose(tp[0:64, :], w2_nat[:, m, :], ident)
            nc.vector.tensor_copy(w2T[:, m, :], tp[0:64, :])

        b1_t = singles.tile([64, 1], F32)
        nc.sync.dma_start(out=b1_t, in_=b1_d)
        b2_t = singles.tile([128, CB], F32)
        for m in range(CB):
            nc.sync.dma_start(out=b2_t[:, m : m + 1], in_=b2_d[128 * m : 128 * (m + 1), :])
        nb2_t = singles.tile([128, CB], F32)
        nc.vector.tensor_scalar(
            out=nb2_t, in0=b2_t, scalar1=-1.0, scalar2=None, op0=ALU.mult
        )

        # one persistent tiny PSUM slot for the SE-gate matvecs:
        # cols 0-3 = pooled_out, col 4 = h (rows 0-63), cols 5-8 = se
        tiny = psum.tile([128, 12], F32, tag="tiny", bufs=1)

        # ---- per-sample pipeline -------------------------------------------
        for rep in range(reps):
          for b in range(B_PER_CORE):
            # 1a/1b. q^T groups + q (prefetched before setup for the very
            # first sample; see top of _emit)
            if rep == 0 and b == 0:
                qts, q = pre_qts, pre_q
            else:
                qts = []
                for j in range(JT):
                    qt4 = qtpool.tile([128, 4, C], F16, tag="qt",
                                      name=f"qt_{rep}_{b}_{j}")
                    nc.sync.dma_start_transpose(
                        out=qt4, in_=x_d[b, :, 512 * j : 512 * (j + 1)]
                    )
                    qts.append(qt4)
                q = qpool.tile([128, CB, N], F16, tag="q", name=f"q_s{rep}_{b}")
                for m in range(CB):
                    nc.sync.dma_start(
                        out=q[:, m, :],
                        in_=x_d[b, 128 * m : 128 * (m + 1), :],
                    )

            # 2. pooled_x partials on DVE (tensor_scalar accum), staggered
            px_part = stats.tile([128, CB, 2], F32, tag="pxp")
            pxscr = stats.tile([128, 2048], F16, tag="pxscr", bufs=1)

            # 3. energy = q @ q.T (fp16, fp32 accumulate), upper-tri blocks
            eps = [
                psum.tile([128, C - 128 * m], F32, tag="bank", bufs=5,
                          name=f"eps_{rep}_{b}_{m}")
                for m in range(CB)
            ]
            for kt in range(KT):
                qt = qts[kt // 4][:, kt % 4, :]
                for m in range(CB):
                    nc.tensor.matmul(
                        eps[m],
                        lhsT=qt[:, 128 * m : 128 * (m + 1)],
                        rhs=qt[:, 128 * m :],
                        start=(kt == 0),
                        stop=(kt == KT - 1),
                    )
                if 12 <= kt < 16:
                    m_, h_ = kt - 12, 0
                elif 16 <= kt < 20:
                    m_, h_ = kt - 16, 1
                else:
                    m_ = None
                if m_ is not None:
                    hsl = slice(2048 * h_, 2048 * (h_ + 1))
                    nc.vector.tensor_scalar(
                        out=pxscr,
                        in0=q[:, m_, hsl],
                        scalar1=1.0,
                        scalar2=0.0,
                        op0=ALU.mult,
                        op1=ALU.add,
                        accum_out=px_part[:, m_, h_ : h_ + 1],
                    )

            px_mean = stats.tile([128, CB], F32, tag="px")
            px_raw = stats.tile([128, CB], F32, tag="pxr")
            nc.vector.tensor_reduce(out=px_raw, in_=px_part, axis=AX.X, op=ALU.add)
            nc.scalar.mul(px_mean, px_raw, 1.0 / N)
            px_h = stats.tile([128, CB], F16, tag="pxb")
            nc.vector.tensor_copy(px_h, px_mean)

            # 4+5 per-block pipeline: evacuate row-block m, mirror its lower
            # blocks, min, exp (fp16 G + fp32 rowsum), then G^T via one
            # crossbar transpose issued right behind the exp on the ACT queue.
            en = gpool.tile([128, CB, C], F32, tag="en")
            nmin = stats.tile([128, CB], F32, tag="nmin")
            G = gpool.tile([128, CB, C], F16, tag="G")
            S = stats.tile([128, CB], F32, tag="S")
            GT = gtpool.tile([128, CB, C], F16, tag="GT")
            gstage = [
                psum.tile([128, C], F16, tag="bank", bufs=5,
                          name=f"gst_{rep}_{b}_{k}")
                for k in range(CB)
            ]
            for m in range(CB):
                # mirrors first: they read row-blocks j<m (already in SBUF),
                # so the PE transposes overlap this block's DVE evacuation
                if m > 0:
                    tps = psum.tile([128, C], F32, tag="tstage", bufs=2)
                    for j in range(m):
                        # block (m, j) = block (j, m)^T
                        nc.tensor.transpose(
                            tps[:, 128 * j : 128 * (j + 1)],
                            en[:, j, 128 * m : 128 * (m + 1)],
                            ident,
                        )
                nc.vector.tensor_copy(en[:, m, 128 * m :], eps[m])
                if m > 0:
                    nc.vector.tensor_copy(en[:, m, : 128 * m], tps[:, : 128 * m])
                nc.vector.tensor_reduce(
                    out=nmin[:, m : m + 1], in_=en[:, m, :], axis=AX.X, op=ALU.min
                )
                nc.scalar.activation(
                    out=G[:, m, :],
                    in_=en[:, m, :],
                    func=AF.Exp,
                    bias=nmin[:, m : m + 1],
                    scale=-1.0,
                    accum_out=S[:, m : m + 1],
                )
                # (G row-block m)^T on the PE (off the DMA pipe, which
                # is busy prefetching the next sample's q^T tiles)
                for k in range(CB):
                    nc.tensor.transpose(
                        gstage[k][:, 128 * m : 128 * (m + 1)],
                        G[:, m, 128 * k : 128 * (k + 1)],
                        identh,
                    )
                for k in range(CB):
                    nc.vector.tensor_copy(
                        GT[:, k, 128 * m : 128 * (m + 1)],
                        gstage[k][:, 128 * m : 128 * (m + 1)],
                    )

            # 6. pooled_out = (G @ px_mean) / S
            for m in range(CB):
                for k in range(CB):
                    nc.tensor.matmul(
                        tiny[:, m : m + 1],
                        lhsT=GT[:, k, 128 * m : 128 * (m + 1)],
                        rhs=px_h[:, k : k + 1],
                        start=(k == 0),
                        stop=(k == CB - 1),
                    )
            recipS = stats.tile([128, CB], F32, tag="rS")
            nc.vector.reciprocal(recipS, S)
            po_mean = stats.tile([128, CB], F32, tag="po")
            for m in range(CB):
                nc.scalar.activation(
                    po_mean[:, m : m + 1], tiny[:, m : m + 1], AF.Copy,
                    scale=recipS[:, m : m + 1],
                )

            # 8a. first mm2 chunk's off-diagonal terms start now and hide the
            # whole SE-gate latency chain below.
            m0banks = [
                psum.tile([128, 512], F32, tag="bank", bufs=5,
                          name=f"po_{rep}_{b}_0_{jj}")
                for jj in range(4)
            ]
            for ki, k in enumerate((1, 2, 3)):
                for jj in range(4):
                    nc.tensor.matmul(
                        m0banks[jj],
                        lhsT=GT[:, k, 0:128],
                        rhs=q[:, k, 512 * jj : 512 * (jj + 1)],
                        start=(ki == 0),
                        stop=False,
                    )

            # 7. SE gate: h = relu(w1@pooled+b1); se = sigmoid(w2@h+b2)
            for k in range(8):
                rhs = px_mean[:, k : k + 1] if k < 4 else po_mean[:, k - 4 : k - 3]
                nc.tensor.matmul(
                    tiny[0:64, 4:5],
                    lhsT=w1T[:, k, :],
                    rhs=rhs,
                    start=(k == 0),
                    stop=(k == 7),
                )
            h_sb = stats.tile([64, 1], F32, tag="h")
            nc.scalar.activation(h_sb, tiny[0:64, 4:5], AF.Relu, bias=b1_t)

            for m in range(CB):
                nc.tensor.matmul(
                    tiny[:, 5 + m : 6 + m],
                    lhsT=w2T[:, m, :],
                    rhs=h_sb,
                    start=True,
                    stop=True,
                )
            # sigmoid via exp so the whole kernel stays in one ACT
            # function-table set (exp_and_others): se = 1/(1+exp(-(z+b2)))
            se_e = stats.tile([128, CB], F32, tag="see")
            se_1 = stats.tile([128, CB], F32, tag="se1")
            se = stats.tile([128, CB], F32, tag="se")
            for m in range(CB):
                nc.scalar.activation(
                    se_e[:, m : m + 1], tiny[:, 5 + m : 6 + m], AF.Exp,
                    bias=nb2_t[:, m : m + 1], scale=-1.0,
                )
            nc.vector.tensor_scalar(
                out=se_1, in0=se_e, scalar1=1.0, scalar2=None, op0=ALU.add
            )
            nc.vector.reciprocal(se, se_1)
            # beta = (1-se)/S ; dvec = se/beta = se*S/(1-se)
            beta0 = stats.tile([128, CB], F32, tag="b0")
            beta = stats.tile([128, CB], F32, tag="b1")
            dvec = stats.tile([128, CB], F32, tag="dv")
            rb0 = stats.tile([128, CB], F32, tag="rb0")
            seS = stats.tile([128, CB], F32, tag="seS")
            nc.vector.tensor_scalar(
                out=beta0, in0=se, scalar1=-1.0, scalar2=1.0, op0=ALU.mult, op1=ALU.add
            )
            nc.vector.tensor_mul(beta, beta0, recipS)
            nc.vector.reciprocal(rb0, beta0)
            nc.vector.tensor_mul(seS, se, S)
            nc.vector.tensor_mul(dvec, seS, rb0)
            # fold diag(dvec) into GT's diagonal blocks:
            # GT[:, m, mcols] += ident * dvec[:, m]
            for m in range(CB):
                nc.vector.scalar_tensor_tensor(
                    out=GT[:, m, 128 * m : 128 * (m + 1)],
                    in0=identh,
                    scalar=dvec[:, m : m + 1],
                    in1=GT[:, m, 128 * m : 128 * (m + 1)],
                    op0=ALU.mult,
                    op1=ALU.add,
                )

            # 8b. y_raw = (G + diag(se/beta)) @ q, k==m last; evacuate with
            # the beta scale split across ACT and DVE, store fp16 from the
            # ACT queue in 2KB/partition chunks.
            def mm2_evac(m, half, banks, split=False):
                fin = outp.tile([128, 4, 512], F16, tag="fin", bufs=5)
                for jj in range(4):
                    if jj % 2 == 0:
                        nc.scalar.activation(
                            fin[:, jj, :], banks[jj], AF.Copy,
                            scale=beta[:, m : m + 1],
                        )
                    else:
                        nc.vector.tensor_scalar(
                            out=fin[:, jj, :], in0=banks[jj],
                            scalar1=beta[:, m : m + 1], scalar2=None,
                            op0=ALU.mult,
                        )
                    if split and jj % 2 == 1:
                        nc.sync.dma_start(
                            out=y_d[b, 128 * m : 128 * (m + 1),
                                    2048 * half + 1024 * (jj // 2) :
                                    2048 * half + 1024 * (jj // 2 + 1)],
                            in_=fin[:, jj - 1 : jj + 1, :],
                        )
                if not split:
                    nc.sync.dma_start(
                        out=y_d[b, 128 * m : 128 * (m + 1),
                                2048 * half : 2048 * (half + 1)],
                        in_=fin,
                    )

            def mm2_evac2(m, half, banks):
                # same evacuation but 2 stores of 1024 for finer overlap
                fin = outp.tile([128, 4, 512], F16, tag="fin", bufs=5)
                for jj in range(4):
                    if jj % 2 == 0:
                        nc.scalar.activation(
                            fin[:, jj, :], banks[jj], AF.Copy,
                            scale=beta[:, m : m + 1],
                        )
                    else:
                        nc.vector.tensor_scalar(
                            out=fin[:, jj, :], in0=banks[jj],
                            scalar1=beta[:, m : m + 1], scalar2=None,
                            op0=ALU.mult,
                        )
                        nc.sync.dma_start(
                            out=y_d[b, 128 * m : 128 * (m + 1),
                                    2048 * half + 1024 * (jj // 2) :
                                    2048 * half + 1024 * (jj // 2 + 1)],
                            in_=fin[:, jj - 1 : jj + 1, :],
                        )

            # finish m=0, half=0 (diagonal term), then evacuate
            for jj in range(4):
                nc.tensor.matmul(
                    m0banks[jj],
                    lhsT=GT[:, 0, 0:128],
                    rhs=q[:, 0, 512 * jj : 512 * (jj + 1)],
                    start=False,
                    stop=True,
                )
            mm2_evac(0, 0, m0banks)

            for m in range(CB):
                korder = [k for k in range(CB) if k != m] + [m]
                for half in range(2):
                    if m == 0 and half == 0:
                        continue
                    j0 = 4 * half
                    banks = [
                        psum.tile([128, 512], F32, tag="bank", bufs=5,
                                  name=f"po_{rep}_{b}_{m}_{j0 + jj}")
                        for jj in range(4)
                    ]
                    for ki, k in enumerate(korder):
                        for jj in range(4):
                            j = j0 + jj
                            nc.tensor.matmul(
                                banks[jj],
                                lhsT=GT[:, k, 128 * m : 128 * (m + 1)],
                                rhs=q[:, k, 512 * j : 512 * (j + 1)],
                                start=(ki == 0),
                                stop=(ki == CB - 1),
                            )
                    mm2_evac(m, half, banks, split=(m == CB - 1 and half == 1))


_NC_CACHE = None


def _get_program():
    global _NC_CACHE
    if _NC_CACHE is None:
        _NC_CACHE = _build_program()
    return _NC_CACHE


def kernel(x, w1, b1, w2, b2, _trace=False):
    x = np.ascontiguousarray(x, dtype=np.float32)
    B, Cc, H, W = x.shape
    assert (B, Cc, H * W) == (B_TOTAL, C, N)
    xr = x.reshape(B, Cc, H * W).astype(np.float16)
    in_maps = []
    for i in range(N_CORES):
        in_maps.append(
            {
                "x": np.ascontiguousarray(xr[B_PER_CORE * i : B_PER_CORE * (i + 1)]),
                "w1": np.ascontiguousarray(w1, dtype=np.float32),
                "b1": np.ascontiguousarray(b1, dtype=np.float32).reshape(64, 1),
                "w2": np.ascontiguousarray(w2, dtype=np.float32),
                "b2": np.ascontiguousarray(b2, dtype=np.float32).reshape(C, 1),
            }
        )
    nc = _get_program()
    res = run_bass_kernel_spmd(nc, in_maps, list(range(N_CORES)), trace=_trace)
    y = np.concatenate([res.results[i]["y"] for i in range(N_CORES)], axis=0)
    out = y.reshape(B, Cc, H, W).astype(np.float32)
    if _trace:
        return out, res
    return out
